# revision 12
# baseline (speedup 1.0000x reference)
"""Trainium2 Bass kernel for nn_Model_10617159155719 (GNN + capsule routing).

Strategy: pure data parallel over the batch dim b=32 across 8 NeuronCores
(4 graphs per core). Per graph, the device computes:
  embeddings-matmul GCN (3 layers) -> attention -> graph-capsule projection
  u_hat -> 3-iteration dynamic routing -> v_full (16x512, diag blocks = v_g)
The tiny class-capsule routing (16 input capsules), margin loss, recon loss
and argmax run on host in float32 numpy (negligible FLOPs).

All device math is fp32. Layouts are chosen so every big op is a PE matmul:
  - adj is symmetric => A_norm = D^-1/2 A D^-1/2 is its own transpose and
    serves directly as matmul weights (lhsT).
  - u_hat (3072x512) and its transpose are both produced by matmuls from
    H'^T (no explicit PE-transpose passes over u).
  - routing einsums:  s = c^T @ u  (diag blocks), t = u @ V_blockdiag via
    u^T as moving operand; b updates accumulate through 16-col transposes.
"""

import os
from contextlib import ExitStack

import numpy as np

B, N = 32, 512
NCORES = 8
NB = B // NCORES          # graphs per core
EMB = 64
GCN_IN = 192              # 3 * 64
HID = 128                 # emb * C
CL = 6                    # C * L channels
ATT = 384                 # emb * C * L
JG, DG = 16, 32
JD = JG * DG              # 512
NT = 4                    # node tiles of 128
PT = CL * NT              # 24 capsule tiles of 128
EPS = 1e-11
NEG = -1e30


# ----------------------------------------------------------------- device ---

def _split_multiwaits(nc):
    """walrus codegen in this toolchain accepts at most ONE sync-wait per
    instruction (setupSyncWait: 'Too many sync wait commands').  Tile emits
    instruction-attached multi-waits; split the extras into single-wait NoOps
    on the same engine immediately before the instruction."""
    from concourse import mybir
    for fn in nc.m.functions:
        for blk in fn.blocks:
            il = blk.instructions
            out = []
            changed = False
            for ins in il:
                si = ins.sync_info
                if si is not None and len(si.on_wait) > 1:
                    waits = list(si.on_wait)
                    for w in waits[:-1]:
                        out.append(mybir.InstNoOp(
                            name=nc.get_next_instruction_name(),
                            engine=ins.engine,
                            sync_info=mybir.SyncInfo(on_wait=[w], on_update=[]),
                            bass_nofuse=True,
                        ))
                    ins.sync_info = mybir.SyncInfo(
                        on_wait=waits[-1:], on_update=list(si.on_update))
                    changed = True
                out.append(ins)
            if changed:
                blk.instructions = out


def _build(nb: int):
    import concourse.bass as bass
    import concourse.tile as tile
    from concourse import mybir

    f32 = mybir.dt.float32
    AF = mybir.ActivationFunctionType
    ALU = mybir.AluOpType
    AX = mybir.AxisListType

    nc = bass.Bass()

    adj_d = nc.declare_dram_parameter("adj", [nb, N, N], f32, isOutput=False)
    ft_d = nc.declare_dram_parameter("featsT", [nb, GCN_IN, N], f32, isOutput=False)
    gw0_d = nc.declare_dram_parameter("gw0", [GCN_IN, HID], f32, isOutput=False)
    gw1_d = nc.declare_dram_parameter("gw1", [HID, HID], f32, isOutput=False)
    gw2_d = nc.declare_dram_parameter("gw2", [HID, HID], f32, isOutput=False)
    gbs_d = nc.declare_dram_parameter("gbs", [3, HID], f32, isOutput=False)
    aw1_d = nc.declare_dram_parameter("aw1", [ATT, ATT], f32, isOutput=False)
    ab1_d = nc.declare_dram_parameter("ab1", [ATT], f32, isOutput=False)
    aw2_d = nc.declare_dram_parameter("aw2", [ATT], f32, isOutput=False)
    wg_d = nc.declare_dram_parameter("wgT", [EMB, CL, JD], f32, isOutput=False)
    outv_d = nc.declare_dram_parameter("outv", [nb, JG, JD], f32, isOutput=True)

    eye128_d = nc.inline_tensor(np.eye(128, dtype=np.float32), "eye128")
    eye16_d = nc.inline_tensor(np.eye(16, dtype=np.float32), "eye16")
    bdm = np.zeros((JG, JD), dtype=np.float32)
    for j in range(JG):
        bdm[j, j * DG:(j + 1) * DG] = 1.0
    bdmask_d = nc.inline_tensor(bdm, "bdmask")

    def bcast_ap(ap, p):
        # partition-broadcast a (1, n)/(n,) dram AP to p partitions
        return bass.AP(tensor=ap.tensor, offset=ap.offset, ap=[[0, p]] + list(ap.ap))

    with tile.TileContext(nc) as tc, ExitStack() as ctx:
        const = ctx.enter_context(tc.tile_pool(name="const", bufs=1))
        ubig = ctx.enter_context(tc.tile_pool(name="ubig", bufs=1))
        abig = ctx.enter_context(tc.tile_pool(name="abig", bufs=1))
        stream = ctx.enter_context(tc.tile_pool(name="stream", bufs=2))
        perb = ctx.enter_context(tc.tile_pool(name="perb", bufs=1))
        small = ctx.enter_context(tc.tile_pool(name="small", bufs=2))
        psA = ctx.enter_context(tc.tile_pool(name="psA", bufs=2, space="PSUM"))
        psS = ctx.enter_context(tc.tile_pool(name="psS", bufs=2, space="PSUM"))
        psB = ctx.enter_context(tc.tile_pool(name="psB", bufs=2, space="PSUM"))
        psbt = ctx.enter_context(tc.tile_pool(name="psbt", bufs=1, space="PSUM"))

        # ---- constants / parameters (loaded once) ----
        eye128 = const.tile([128, 128], f32)
        nc.gpsimd.dma_start(out=eye128, in_=eye128_d[:, :])
        eye16 = const.tile([16, 16], f32)
        nc.gpsimd.dma_start(out=eye16, in_=eye16_d[:, :])
        bdmask = const.tile([JG, JD], f32)
        nc.gpsimd.dma_start(out=bdmask, in_=bdmask_d[:, :])
        ones_row = const.tile([1, 128], f32)
        nc.vector.memset(ones_row, 1.0)
        cunif = const.tile([128, JG], f32)
        nc.vector.memset(cunif, 1.0 / JG)
        eps16 = const.tile([16, 1], f32)
        nc.vector.memset(eps16, EPS)

        w0a = const.tile([128, HID], f32)
        nc.gpsimd.dma_start(out=w0a, in_=gw0_d[0:128, :])
        w0b = const.tile([64, HID], f32)
        nc.gpsimd.dma_start(out=w0b, in_=gw0_d[128:192, :])
        w1 = const.tile([128, HID], f32)
        nc.gpsimd.dma_start(out=w1, in_=gw1_d[:, :])
        w2 = const.tile([128, HID], f32)
        nc.gpsimd.dma_start(out=w2, in_=gw2_d[:, :])
        bias_bc = const.tile([128, 3, HID], f32)
        for l in range(3):
            nc.gpsimd.dma_start(out=bias_bc[:, l, :], in_=bcast_ap(gbs_d[l, :], 128))
        aw1_sb = const.tile([128, 3, ATT], f32)
        nc.gpsimd.dma_start(out=aw1_sb, in_=aw1_d.rearrange("(t p) f -> p t f", p=128))
        ab1_sb = const.tile([128, 3], f32)
        nc.gpsimd.dma_start(out=ab1_sb, in_=ab1_d.rearrange("(t p) -> p t", p=128))
        aw2_sb = const.tile([128, 3], f32)
        nc.gpsimd.dma_start(out=aw2_sb, in_=aw2_d.rearrange("(t p) -> p t", p=128))
        wg_sb = const.tile([EMB, CL, JD], f32)
        nc.gpsimd.dma_start(out=wg_sb, in_=wg_d[:, :, :])

        for bi in range(nb):
            # ---- load & normalize adjacency ----
            adj = abig.tile([128, NT, N], f32, tag="adj")
            nc.sync.dma_start(out=adj, in_=adj_d[bi].rearrange("(t p) j -> p t j", p=128))
            deg = small.tile([128, NT], f32, tag="deg")
            nc.vector.tensor_reduce(deg, adj, axis=AX.X, op=ALU.add)
            masks = small.tile([128, NT], f32, tag="masks")
            nc.vector.tensor_reduce(masks, adj, axis=AX.X, op=ALU.max)
            ispos = small.tile([128, NT], f32, tag="ispos")
            nc.vector.tensor_scalar(ispos, deg, 0.0, None, op0=ALU.is_gt)
            onem = small.tile([128, NT], f32, tag="onem")
            nc.vector.tensor_scalar(onem, ispos, -1.0, 1.0, op0=ALU.mult, op1=ALU.add)
            degs = small.tile([128, NT], f32, tag="degs")
            nc.vector.tensor_add(degs, deg, onem)
            nc.scalar.activation(degs, degs, AF.Sqrt)
            nc.vector.reciprocal(degs, degs)
            dinv = small.tile([128, NT], f32, tag="dinv")
            nc.vector.tensor_mul(dinv, degs, ispos)

            # rows (1,512): transpose dinv & masks via PE
            prow = psS.tile([1, N], f32, tag="pS")
            for t in range(NT):
                nc.tensor.transpose(prow[0:1, t * 128:(t + 1) * 128], dinv[:, t:t + 1], eye128)
            dinv_row = perb.tile([1, N], f32, tag="dinvrow")
            nc.vector.tensor_copy(dinv_row, prow)
            prow2 = psS.tile([1, N], f32, tag="pS")
            for t in range(NT):
                nc.tensor.transpose(prow2[0:1, t * 128:(t + 1) * 128], masks[:, t:t + 1], eye128)
            masks_row = perb.tile([1, N], f32, tag="masksrow")
            nc.vector.tensor_copy(masks_row, prow2)

            # num_nodes and friends
            nn_t = small.tile([1, 1], f32, tag="nn")
            nc.vector.tensor_reduce(nn_t, masks_row, axis=AX.X, op=ALU.add)
            nnr_t = small.tile([1, 1], f32, tag="nnr")
            nc.vector.reciprocal(nnr_t, nn_t)
            nn2row = small.tile([1, 2], f32, tag="nn2row")
            nc.vector.tensor_copy(nn2row[:, 0:1], nnr_t)
            nc.vector.tensor_mul(nn2row[:, 1:2], nnr_t, nnr_t)
            pnn = psA.tile([128, 2], f32, tag="pA")
            nc.tensor.matmul(pnn, ones_row, nn2row, start=True, stop=True)
            nncol = small.tile([128, 2], f32, tag="nncol")
            nc.vector.tensor_copy(nncol, pnn)

            # attention -inf mask row
            ipr = perb.tile([1, N], f32, tag="ipr")
            nc.vector.tensor_scalar(ipr, masks_row, 0.0, None, op0=ALU.is_gt)
            mneg = perb.tile([1, N], f32, tag="mneg")
            nc.vector.tensor_scalar(mneg, ipr, -NEG, NEG, op0=ALU.mult, op1=ALU.add)

            # A_norm = dinv_col * A * dinv_row   (in place; symmetric)
            pbr = psA.tile([128, N], f32, tag="pA")
            nc.tensor.matmul(pbr, ones_row, dinv_row, start=True, stop=True)
            for t in range(NT):
                nc.vector.tensor_scalar(adj[:, t, :], adj[:, t, :], dinv[:, t:t + 1], None, op0=ALU.mult)
                nc.vector.tensor_mul(adj[:, t, :], adj[:, t, :], pbr)

            # ---- node features (transposed) ----
            ft0 = stream.tile([128, N], f32, tag="ft0")
            nc.sync.dma_start(out=ft0, in_=ft_d[bi, 0:128, :])
            ft1 = stream.tile([64, N], f32, tag="ft1")
            nc.sync.dma_start(out=ft1, in_=ft_d[bi, 128:192, :])

            # ---- GCN layers ----
            hts = []
            for l in range(3):
                if l == 0:
                    ktiles = [(ft0, w0a), (ft1, w0b)]
                else:
                    ktiles = [(hts[l - 1], w1 if l == 1 else w2)]
                # P1: Z = X @ W + b   (n-part, h-free)
                y = stream.tile([128, NT, HID], f32, tag="y")
                for mt in range(NT):
                    pz = psA.tile([128, HID], f32, tag="pA")
                    for ki, (xt, wt) in enumerate(ktiles):
                        nc.tensor.matmul(
                            pz, xt[:, mt * 128:(mt + 1) * 128], wt,
                            start=(ki == 0), stop=(ki == len(ktiles) - 1))
                    nc.vector.tensor_add(y[:, mt, :], pz, bias_bc[:, l, :])
                # P2: h = tanh(A_norm @ Z)
                h = stream.tile([128, NT, HID], f32, tag="h")
                for mt in range(NT):
                    ph = psA.tile([128, HID], f32, tag="pA")
                    for kt in range(NT):
                        nc.tensor.matmul(
                            ph, adj[:, kt, mt * 128:(mt + 1) * 128], y[:, kt, :],
                            start=(kt == 0), stop=(kt == NT - 1))
                    nc.scalar.activation(h[:, mt, :], ph, AF.Tanh)
                # transpose h -> hT (128, 512)
                ht = perb.tile([128, N], f32, tag=f"hT{l}")
                for mt in range(NT):
                    pt_ = psA.tile([128, 128], f32, tag="pA")
                    nc.tensor.transpose(pt_, h[:, mt, :], eye128)
                    nc.scalar.copy(ht[:, mt * 128:(mt + 1) * 128], pt_)
                hts.append(ht)

            # ---- attention ----
            a1t = []
            for m3 in range(3):
                pa = psB.tile([128, N], f32, tag="pB")
                for k3 in range(3):
                    nc.tensor.matmul(
                        pa, aw1_sb[:, k3, m3 * 128:(m3 + 1) * 128], hts[k3],
                        start=(k3 == 0), stop=(k3 == 2))
                at = perb.tile([128, N], f32, tag=f"a1t{m3}")
                nc.scalar.activation(at, pa, AF.Tanh, bias=ab1_sb[:, m3:m3 + 1])
                a1t.append(at)
            pat = psS.tile([1, N], f32, tag="pS")
            for m3 in range(3):
                nc.tensor.matmul(pat, aw2_sb[:, m3:m3 + 1], a1t[m3],
                                 start=(m3 == 0), stop=(m3 == 2))
            attm = perb.tile([1, N], f32, tag="attm")
            nc.vector.tensor_add(attm, pat, mneg)
            nc.scalar.activation(attm, attm, AF.Exp)
            asum = small.tile([1, 1], f32, tag="asum")
            nc.vector.tensor_reduce(asum, attm, axis=AX.X, op=ALU.add)
            nc.vector.reciprocal(asum, asum)
            attrow = perb.tile([1, N], f32, tag="attrow")
            nc.vector.tensor_scalar(attrow, attm, asum[:, 0:1], nn_t[:, 0:1],
                                    op0=ALU.mult, op1=ALU.mult)
            pab = psB.tile([128, N], f32, tag="pB")
            nc.tensor.matmul(pab, ones_row, attrow, start=True, stop=True)
            hsc = []
            hsch = []   # odd-channel halves shifted to base partition 0 (DMA)
            for l in range(3):
                hs = perb.tile([128, N], f32, tag=f"hsc{l}")
                nc.vector.tensor_mul(hs, hts[l], pab)
                hsc.append(hs)
                hsh = perb.tile([64, N], f32, tag=f"hsch{l}")
                nc.sync.dma_start(out=hsh, in_=hs[64:128, :])
                hsch.append(hsh)

            # ---- u_hat (24 tiles of (128 nodes, 512 jd)) and its transpose ----
            u_sb = ubig.tile([128, PT, JD], f32, tag="u")
            uT_sb = ubig.tile([128, PT, JD], f32, tag="uT")
            for c in range(CL):
                src = hsc[c // 2][0:64, :] if c % 2 == 0 else hsch[c // 2]
                for mt in range(NT):
                    pu = psB.tile([128, JD], f32, tag="pB")
                    nc.tensor.matmul(
                        pu, src[:, mt * 128:(mt + 1) * 128],
                        wg_sb[:, c, :], start=True, stop=True)
                    if mt % 2 == 0:
                        nc.scalar.copy(u_sb[:, c * NT + mt, :], pu)
                    else:
                        nc.vector.tensor_copy(u_sb[:, c * NT + mt, :], pu)
                for st in range(NT):
                    pu = psB.tile([128, JD], f32, tag="pB")
                    nc.tensor.matmul(
                        pu, wg_sb[:, c, st * 128:(st + 1) * 128],
                        src, start=True, stop=True)
                    if st % 2 == 0:
                        nc.vector.tensor_copy(uT_sb[:, c * NT + st, :], pu)
                    else:
                        nc.scalar.copy(uT_sb[:, c * NT + st, :], pu)

            # ---- dynamic routing (3 iters) ----
            b0sb = None   # iter-0 logits, copied to SBUF
            bt1 = None    # iter-1 logit increments (PSUM)
            for it in range(3):
                if it == 0:
                    c_of = lambda pt: cunif
                else:
                    if it == 1:
                        bsum = b0sb
                    else:
                        bsum = perb.tile([128, PT, JG], f32, tag="bsum")
                        nc.vector.tensor_add(bsum, b0sb, bt1)
                    eb = perb.tile([128, PT, JG], f32, tag="eb")
                    nc.scalar.activation(eb, bsum, AF.Exp)
                    esum = small.tile([128, PT], f32, tag="esum")
                    nc.vector.tensor_reduce(esum, eb, axis=AX.X, op=ALU.add)
                    nc.vector.reciprocal(esum, esum)
                    csb = perb.tile([128, PT, JG], f32, tag="csb")
                    for pt in range(PT):
                        nc.vector.tensor_scalar(
                            csb[:, pt, :], eb[:, pt, :], esum[:, pt:pt + 1], None,
                            op0=ALU.mult)
                    c_of = lambda pt: csb[:, pt, :]

                ps_s = psS.tile([JG, JD], f32, tag="pS")
                for pt in range(PT):
                    nc.tensor.matmul(ps_s, c_of(pt), u_sb[:, pt, :],
                                     start=(pt == 0), stop=(pt == PT - 1))

                # squash (reference scales s by 1/num_nodes first)
                s2 = perb.tile([JG, JG, DG], f32, tag="s2")
                nc.scalar.activation(s2, ps_s.rearrange("j (k d) -> j k d", d=DG), AF.Square)
                r16 = small.tile([JG, JG], f32, tag="r16")
                nc.vector.tensor_reduce(r16, s2, axis=AX.X, op=ALU.add)
                nc.vector.tensor_mul(r16, r16, eye16)
                sq = small.tile([JG, 1], f32, tag="sq")
                nc.vector.tensor_reduce(sq, r16, axis=AX.X, op=ALU.add)
                nc.vector.tensor_mul(sq, sq, nncol[0:JG, 1:2])     # sq/nn^2
                d1 = small.tile([JG, 1], f32, tag="d1")
                nc.vector.tensor_scalar_add(d1, sq, 1.0)
                nc.vector.reciprocal(d1, d1)
                num = small.tile([JG, 1], f32, tag="num")
                nc.vector.tensor_mul(num, sq, d1)                  # sq/(1+sq)
                r2 = small.tile([JG, 1], f32, tag="r2")
                nc.scalar.activation(r2, sq, AF.Sqrt, bias=eps16[:, 0:1])
                nc.vector.reciprocal(r2, r2)                       # 1/sqrt(sq+eps)
                fs = small.tile([JG, 1], f32, tag="fs")
                nc.vector.tensor_mul(fs, num, r2)
                nc.vector.tensor_mul(fs, fs, nncol[0:JG, 0:1])     # * 1/nn
                vfull = perb.tile([JG, JD], f32, tag="vfull")
                nc.vector.tensor_scalar(vfull, ps_s, fs[:, 0:1], None, op0=ALU.mult)

                if it == 2:
                    nc.sync.dma_start(out=outv_d[bi], in_=vfull)
                    break

                # V block-diag (512,16) from v
                vbd = perb.tile([JG, JD], f32, tag="vbd")
                nc.vector.tensor_mul(vbd, vfull, bdmask)
                V = perb.tile([128, NT, JG], f32, tag="V")
                for st in range(NT):
                    pv = psA.tile([128, JG], f32, tag="pA")
                    nc.tensor.transpose(pv, vbd[:, st * 128:(st + 1) * 128], eye16)
                    nc.vector.tensor_copy(V[:, st, :], pv)

                # t = u . v  then transpose-accumulate into b logits
                btile = psbt.tile([128, PT, JG], f32, tag="bt")
                for c in range(CL):
                    ptc = psS.tile([JG, N], f32, tag="pS")
                    for st in range(NT):
                        nc.tensor.matmul(ptc, V[:, st, :], uT_sb[:, c * NT + st, :],
                                         start=(st == 0), stop=(st == NT - 1))
                    tsb = perb.tile([JG, N], f32, tag="tsb")
                    if c % 2 == 0:
                        nc.scalar.copy(tsb, ptc)
                    else:
                        nc.vector.tensor_copy(tsb, ptc)
                    for mt in range(NT):
                        nc.tensor.transpose(btile[:, c * NT + mt, :],
                                            tsb[:, mt * 128:(mt + 1) * 128], eye16)
                if it == 0:
                    b0sb = perb.tile([128, PT, JG], f32, tag="b0sb")
                    nc.vector.tensor_copy(b0sb, btile)
                else:
                    bt1 = btile

    _split_multiwaits(nc)
    return nc


_PROG_CACHE = {}


def _get_prog(nb: int):
    if nb not in _PROG_CACHE:
        _PROG_CACHE[nb] = _build(nb)
    return _PROG_CACHE[nb]


# ------------------------------------------------------------------- host ---

def _softmax_np(x, axis):
    x = x - np.max(x, axis=axis, keepdims=True)
    e = np.exp(x)
    return e / np.sum(e, axis=axis, keepdims=True)


def _squash_np(s):
    sq = np.sum(s * s, axis=-1, keepdims=True)
    return (sq / (1.0 + sq)) * s / np.sqrt(sq + np.float32(EPS))


def _routing_np(u_hat, scale, num_iters=3):
    # u_hat: (b, P, J, D) float32
    b, P, J, D = u_hat.shape
    b_ij = np.zeros((b, P, J), np.float32)
    v = None
    for _ in range(num_iters):
        c = _softmax_np(b_ij, axis=-1)
        s = np.einsum('bpj,bpjd->bjd', c, u_hat).astype(np.float32) / scale
        v = _squash_np(s)
        b_ij = b_ij + np.einsum('bpjd,bjd->bpj', u_hat, v).astype(np.float32)
    return v


def _host_inputs(inputs):
    inp = {k: np.asarray(v) for k, v in inputs.items()}
    emb = [inp['emb0'], inp['emb1'], inp['emb2']]
    nodes = [inp['nodes0'].astype(np.int64), inp['nodes1'].astype(np.int64),
             inp['nodes2'].astype(np.int64)]
    feats = np.concatenate([emb[i][nodes[i]] for i in range(3)], axis=-1)  # (b,n,192)
    featsT = np.ascontiguousarray(np.transpose(feats, (0, 2, 1)).astype(np.float32))
    gbs = np.stack([inp['gb0'], inp['gb1'], inp['gb2']]).astype(np.float32)
    wgT = np.ascontiguousarray(
        np.transpose(inp['Wg'].reshape(CL, EMB, JD), (1, 0, 2)).astype(np.float32))
    per_core = []
    for k in range(NCORES):
        sl = slice(k * NB, (k + 1) * NB)
        per_core.append({
            'adj': np.ascontiguousarray(inp['adj'][sl].astype(np.float32)),
            'featsT': featsT[sl],
            'gw0': inp['gw0'].astype(np.float32),
            'gw1': inp['gw1'].astype(np.float32),
            'gw2': inp['gw2'].astype(np.float32),
            'gbs': gbs,
            'aw1': inp['aw1'].astype(np.float32),
            'ab1': inp['ab1'].astype(np.float32),
            'aw2': inp['aw2'].reshape(ATT).astype(np.float32),
            'wgT': wgT,
        })
    return inp, per_core


def _run_device(per_core, trace=False):
    from concourse.bass_utils import run_bass_kernel_spmd
    nc = _get_prog(NB)
    res = run_bass_kernel_spmd(nc, per_core, list(range(NCORES)), trace=trace)
    vfull = np.concatenate([res.results[k]['outv'] for k in range(NCORES)], axis=0)
    return vfull, res


def _host_tail(inp, vfull):
    # vfull: (B, 16, 512); diag blocks are v_g
    idx = np.arange(JG)
    v_g = np.stack([vfull[:, j, j * DG:(j + 1) * DG] for j in idx], axis=1)  # (B,16,32)
    v_g = v_g.astype(np.float32)

    Wc = np.asarray(inp['Wc']).astype(np.float32)        # (16, 32, 6, 32)
    label = np.asarray(inp['label']).astype(np.int64)
    recon = np.asarray(inp['reconstructs']).astype(np.float32)
    ncls = Wc.shape[2]

    u2 = np.einsum('bci,cijd->bcjd', v_g, Wc).astype(np.float32)  # (B,16,6,32)
    v_c = _routing_np(u2, np.float32(1.0))                        # (B,6,32)

    v_mag = np.sqrt(np.sum(v_c * v_c, axis=2))                    # (B,6)
    pred = np.argmax(v_mag, axis=1).astype(np.int32)
    lambda_val, reg_scale = np.float32(0.5), np.float32(0.1)
    max_l = np.maximum(np.float32(0.9) - v_mag, 0.0) ** 2
    max_r = np.maximum(v_mag - np.float32(0.1), 0.0) ** 2
    T_c = np.eye(ncls, dtype=np.float32)[label]
    L_c = np.sum(T_c * max_l + lambda_val * (1.0 - T_c) * max_r, axis=1)
    margin_loss = np.float32(np.mean(L_c))

    capsule_masked = np.sum(v_c * T_c[..., None], axis=1)         # (B,32)
    rw1 = np.asarray(inp['rw1']).astype(np.float32)
    rb1 = np.asarray(inp['rb1']).astype(np.float32)
    rw2 = np.asarray(inp['rw2']).astype(np.float32)
    rb2 = np.asarray(inp['rb2']).astype(np.float32)
    hidr = np.maximum(capsule_masked @ rw1 + rb1, 0.0)
    rec = 1.0 / (1.0 + np.exp(-(hidr @ rw2 + rb2)))
    neg_ind = (recon < 1e-05).astype(np.float32)
    pos_ind = 1.0 - neg_ind
    rec_val = recon / (np.max(recon, axis=1, keepdims=True) + np.float32(EPS))
    diff = (rec - rec_val).astype(np.float32) ** 2
    recon_loss = np.float32(np.mean(np.max(diff * pos_ind, axis=-1)
                                    + np.max(diff * neg_ind, axis=-1)))
    loss = np.float32(margin_loss + recon_loss * reg_scale)
    return (v_c.astype(np.float32), loss, margin_loss, recon_loss, pred)


def kernel(**inputs):
    inp, per_core = _host_inputs(inputs)
    vfull, _ = _run_device(per_core)
    return _host_tail(inp, vfull)
